# revision 2
# baseline (speedup 1.0000x reference)
"""Depth-aware 3x3 conv on 8 trn2 cores. v4: PE-broadcast fd for two
tap-pairs; one consumed straight from PSUM by DVE, one evicted via ACT.
Engine budget leveled across PE/DVE/ACT/Pool/DMA.

fd packed [72 = 9 taps x 8 segs, 512]. For a tap-adjacent pair (ta, ta+1),
rows [ta*8, ta*8+16) hold both taps' fd. A K=16 matmul with a selector
lhsT sel_j [16, 128] (col c<64 -> e_j, c>=64 -> e_{8+j}) replicates
fd[ta, seg j] to partitions 0-63 and fd[tb, seg j] to 64-127 in PSUM;
ACT evicts to a [128, 4096] fp16 fr tile. Pairs (0,1) and (7,8) go via PE,
pairs (2,3) and (5,6) via the DRAM scatter + stride-0 replicate DMA path.
x2 upper halves are built by GPSIMD copies of the DMA'd lower halves.
"""
import numpy as np

import concourse.bacc as bacc
import concourse.bass as bass
import concourse.mybir as mybir
import concourse.tile as tile
from concourse.bass_utils import run_bass_kernel_spmd

F16 = mybir.dt.float16
F32 = mybir.dt.float32

B, C, H, W = 8, 64, 256, 256
Hp, Wp = H + 2, W + 2
NP = Hp * Wp
ALPHA = 8.3

CH = 4096
NCHUNK = -(-NP // CH)
WT = CH + 520
XTW = WT + 8
SEG, SEGW = 8, CH // 8

XSL, XSH = 512, 4608
DSL, DSH = 512, 4608
XW = XSL + NP + XSH
DW = DSL + NP + DSH
OUTW = NCHUNK * CH

NREG = 4
FDW = NREG * CH

DELTA = [(kh - 1) * Wp + (kw - 1) for kh in range(3) for kw in range(3)]
# (ta, tb, x_tile, mode): mode 'pe' = PE broadcast, 'dma' = scatter+replicate
PAIRS = [(0, 1, 0, "pe_dve"), (7, 8, 0, "pe_act"), (2, 3, 1, "dma"),
         (5, 6, 1, "dma")]
UPPER_SHIFT = [1, 256]
# fdp packed-row start per tap; PE pairs at partition bases 0 and 32 (matmul
# rhs base must be 0/32/64 and match lhsT's base -> sel duplicated at 32-47)
ROW = {0: 0, 1: 8, 2: 16, 3: 24, 7: 32, 8: 40, 5: 48, 6: 56, 4: 64}


def build_nc(repeat=1):
    nc = bacc.Bacc("TRN2", target_bir_lowering=False, debug=False, num_devices=8)
    x_line = nc.declare_dram_parameter("x_line", [C, XW], F16, isOutput=False)
    d_line = nc.declare_dram_parameter("d_line", [1, DW], F32, isOutput=False)
    wts = nc.declare_dram_parameter("wts", [128, 5 * 64], F16, isOutput=False)
    sel = nc.declare_dram_parameter("sel", [48, 8 * 128], F16, isOutput=False)
    bias = nc.declare_dram_parameter("bias", [64, 1], F32, isOutput=False)
    out_l = nc.declare_dram_parameter("out_line", [C, OUTW], F16, isOutput=True)

    x_t = x_line.ap().tensor
    d_t = d_line.ap().tensor
    fd_dram = nc.dram_tensor("fd_scratch", [9, FDW], F16)
    fd_t = fd_dram.ap().tensor

    with tile.TileContext(nc) as tc:
        with (
            tc.tile_pool(name="const", bufs=1) as cpool,
            tc.tile_pool(name="xt", bufs=2) as xpool,
            tc.tile_pool(name="fdgen", bufs=3) as gpool,
            tc.tile_pool(name="frep", bufs=5) as fpool,
            tc.tile_pool(name="mmod", bufs=8) as mpool,
            tc.tile_pool(name="ost", bufs=2) as opool,
            tc.tile_pool(name="ps", bufs=4, space="PSUM") as pspool,
            tc.tile_pool(name="psf", bufs=4, space="PSUM") as psfpool,
        ):
            wt_sb = cpool.tile([128, 5 * 64], F16, tag="w")
            nc.sync.dma_start(wt_sb[:], wts[:])
            sel_sb = cpool.tile([48, 8 * 128], F16, tag="s")
            nc.sync.dma_start(sel_sb[:], sel[:])
            bias_sb = cpool.tile([64, 1], F32, tag="b")
            nc.sync.dma_start(bias_sb[:], bias[:])

            for rep in range(repeat):
                for i in range(NCHUNK):
                    q0 = i * CH
                    xbase = XSL + q0 - 260
                    reg = (i % NREG) * CH
                    # ---- x tiles: lowers from HBM, uppers via Pool copies
                    xt0 = xpool.tile([128, XTW], F16, tag="x0")
                    xt1 = xpool.tile([128, WT], F16, tag="x1")
                    nc.sync.dma_start(
                        xt0[0:64, :],
                        bass.AP(x_t, xbase, [[XW, 64], [1, XTW]]))
                    nc.sync.dma_start(
                        xt1[0:64, :],
                        bass.AP(x_t, xbase, [[XW, 64], [1, WT]]))
                    nc.vector.tensor_copy(xt0[64:128, 1:4614],
                                          xt0[0:64, 2:4615])
                    nc.gpsimd.tensor_copy(xt1[64:128, 3:4357],
                                          xt1[0:64, 259:4613])
                    xts = [xt0, xt1]

                    # ---- fd generation (packed [72, 512], ROW order)
                    dp = gpool.tile([72, SEGW], F32, tag="dp")
                    for (ta, tb, ti, mode) in PAIRS:
                        rb = ROW[ta]
                        nc.gpsimd.dma_start(
                            dp[rb:rb + 16, :],
                            bass.AP(d_t, DSL + q0 + DELTA[ta],
                                    [[DELTA[tb] - DELTA[ta], 2],
                                     [SEGW, SEG], [1, SEGW]]))
                    nc.gpsimd.dma_start(
                        dp[ROW[4]:ROW[4] + SEG, :],
                        bass.AP(d_t, DSL + q0, [[SEGW, SEG], [1, SEGW]]))
                    dc = gpool.tile([72, SEGW], F32, tag="dc")
                    nc.gpsimd.dma_start(
                        dc[:],
                        bass.AP(d_t, DSL + q0,
                                [[0, 9], [SEGW, SEG], [1, SEGW]]))
                    df = gpool.tile([72, SEGW], F32, tag="df")
                    nc.vector.tensor_tensor(df[:], dp[:], dc[:],
                                            mybir.AluOpType.subtract)
                    da = gpool.tile([72, SEGW], F32, tag="da")
                    nc.scalar.activation(da[:], df[:],
                                         mybir.ActivationFunctionType.Abs)
                    fdp = gpool.tile([72, SEGW], F16, tag="fdp")
                    nc.scalar.activation(fdp[:], da[:],
                                         mybir.ActivationFunctionType.Exp,
                                         scale=-ALPHA)
                    # scatter only the DMA pairs' taps
                    for (ta, tb, ti, mode) in PAIRS:
                        if mode != "dma":
                            continue
                        for t9 in (ta, tb):
                            nc.gpsimd.dma_start(
                                bass.AP(fd_t, t9 * FDW + reg,
                                        [[SEGW, SEG], [1, SEGW]]),
                                fdp[ROW[t9]:ROW[t9] + SEG, :])

                    # ---- per pair: fd replicate + modulate ----
                    mts = []
                    for (ta, tb, ti, mode) in PAIRS:
                        mt = mpool.tile([128, CH], F16, tag="m")
                        m0 = 260 + DELTA[ta]
                        if mode == "pe_dve":
                            # broadcast to PSUM; DVE modulates each block
                            # straight from PSUM (1x mode, no evict)
                            rb = ROW[ta]
                            for j in range(SEG):
                                psf = psfpool.tile([128, SEGW], F32)
                                nc.tensor.matmul(
                                    psf[:],
                                    sel_sb[rb:rb + 16,
                                           j * 128:(j + 1) * 128],
                                    fdp[rb:rb + 16, :],
                                    start=True, stop=True)
                                u = j * SEGW
                                nc.vector.tensor_tensor(
                                    mt[:, u:u + SEGW],
                                    xts[ti][:, m0 + u:m0 + u + SEGW],
                                    psf[:], mybir.AluOpType.mult)
                        else:
                            fr = fpool.tile([128, CH], F16, tag="fr")
                            if mode == "dma":
                                nc.sync.dma_start(
                                    fr[0:64, :],
                                    bass.AP(fd_t, ta * FDW + reg,
                                            [[0, 64], [1, CH]]))
                                nc.sync.dma_start(
                                    fr[64:128, :],
                                    bass.AP(fd_t, tb * FDW + reg,
                                            [[0, 64], [1, CH]]))
                            else:  # pe_act
                                rb = ROW[ta]
                                for j in range(SEG):
                                    psf = psfpool.tile([128, SEGW], F32)
                                    nc.tensor.matmul(
                                        psf[:],
                                        sel_sb[rb:rb + 16,
                                               j * 128:(j + 1) * 128],
                                        fdp[rb:rb + 16, :],
                                        start=True, stop=True)
                                    nc.scalar.activation(
                                        fr[:, j * SEGW:(j + 1) * SEGW],
                                        psf[:],
                                        mybir.ActivationFunctionType.Identity,
                                        scale=1.0)
                            nc.vector.tensor_tensor(
                                mt[:], xts[ti][:, m0:m0 + CH], fr[:],
                                mybir.AluOpType.mult)
                        mts.append(mt)

                    # ---- matmuls + eviction ----
                    ost = opool.tile([64, CH], F16, tag="o")
                    for j in range(CH // 512):
                        ps = pspool.tile([64, 512], F32)
                        for g in range(4):
                            nc.tensor.matmul(
                                ps[:], wt_sb[:, g * 64:(g + 1) * 64],
                                mts[g][:, j * 512:(j + 1) * 512],
                                start=(g == 0), stop=False)
                        nc.tensor.matmul(
                            ps[:], wt_sb[0:64, 256:320],
                            xts[0][0:64, 260 + j * 512: 260 + (j + 1) * 512],
                            start=False, stop=True)
                        nc.scalar.activation(
                            ost[:, j * 512:(j + 1) * 512], ps[:],
                            mybir.ActivationFunctionType.Identity,
                            bias=bias_sb[:], scale=1.0)
                    nc.gpsimd.dma_start(out_l[:, q0:q0 + CH], ost[:])
    nc.compile()
    return nc


_NC_CACHE = None


def _get_nc():
    global _NC_CACHE
    if _NC_CACHE is None:
        _NC_CACHE = build_nc()
    return _NC_CACHE


def make_sel():
    s = np.zeros((48, 8 * 128), np.float16)
    for j in range(8):
        for rb in (0, 32):
            s[rb + j, j * 128:j * 128 + 64] = 1.0
            s[rb + 8 + j, j * 128 + 64:(j + 1) * 128] = 1.0
    return s


def kernel(x, depth, weight, bias):
    x = np.asarray(x, dtype=np.float32)
    depth = np.asarray(depth, dtype=np.float32)
    weight = np.asarray(weight, dtype=np.float32)
    bias_np = np.asarray(bias, dtype=np.float32)

    xl = np.zeros((B, C, XW), np.float16)
    xpad = np.zeros((B, C, Hp, Wp), np.float32)
    xpad[:, :, 1:257, 1:257] = x
    xl[:, :, XSL:XSL + NP] = xpad.reshape(B, C, NP).astype(np.float16)

    dl = np.zeros((B, 1, DW), np.float32)
    dpad = np.zeros((B, Hp, Wp), np.float32)
    dpad[:, 1:257, 1:257] = depth[:, 0]
    dl[:, 0, DSL:DSL + NP] = dpad.reshape(B, NP)

    wts = np.zeros((128, 5 * 64), np.float16)
    for g, (ta, tb, _, _) in enumerate(PAIRS):
        wts[0:64, g * 64:(g + 1) * 64] = \
            weight[:, :, ta // 3, ta % 3].T.astype(np.float16)
        wts[64:128, g * 64:(g + 1) * 64] = \
            weight[:, :, tb // 3, tb % 3].T.astype(np.float16)
    wts[0:64, 256:320] = weight[:, :, 1, 1].T.astype(np.float16)

    bias_col = bias_np.reshape(64, 1)
    sel = make_sel()

    nc = _get_nc()
    in_maps = [
        {"x_line": xl[b], "d_line": dl[b], "wts": wts, "sel": sel,
         "bias": bias_col}
        for b in range(B)
    ]
    res = run_bass_kernel_spmd(nc, in_maps, list(range(B)))

    out = np.empty((B, C, H, W), np.float32)
    for b in range(B):
        ol = res.results[b]["out_line"][:, :NP].astype(np.float32)
        out[b] = ol.reshape(C, Hp, Wp)[:, 1:257, 1:257]
    return out
